# revision 7
# baseline (speedup 1.0000x reference)
"""Trainium2 Bass kernel for nn_Attn: additive-attention scores + softmax.

Reference computation (S=512, B=64, H=1024):
    e = relu(concat([hidden bcast, enc], -1) @ Wa^T + ba)      # (S,B,H)
    score = (log(S)/sqrt(H)) * (e @ Ws^T)[...,0]               # (S,B)
    attn = softmax(score.T + pe  with seq_mask -> -1e12, axis=S)  # (B,1,S)

Strategy: data-parallel over B across 8 cores (8 batches each). The concat
splits algebraically: e = relu(enc @ Wa2^T + c[b]) with c = hidden @ Wa1^T + ba
computed once per batch (tiny). Per core the big matmul is (8*512, 1024) @
(1024, 1024), done in e^T orientation (h on partitions, s on free) so the
per-batch bias c fuses into the ACT relu as a per-partition bias and the Ws
reduction is an M=1 matmul on the tensor engine. All matmuls use float32r
(full-rate fp32, ~tf32 mantissa). Host side only reshapes/transposes inputs.
"""
import math
import sys

sys.path.insert(0, "/opt/trn_rl_repo")

import numpy as np

import concourse.bacc as bacc
import concourse.bass as bass
import concourse.mybir as mybir
import concourse.tile as tile
from concourse.bass_utils import run_bass_kernel_spmd

S, B, H = 512, 64, 1024
NCORES = 8
BLOC = B // NCORES          # 8 batches per core
KT = H // 128               # 8 contraction tiles
HT = H // 128               # 8 h-output tiles
SCALE = math.log(S) / math.sqrt(H)

F32R = mybir.dt.float32r
F32 = mybir.dt.float32
U8 = mybir.dt.uint8
AF = mybir.ActivationFunctionType


def build_nc():
    nc = bacc.Bacc("TRN2", target_bir_lowering=False, debug=False,
                   num_devices=NCORES)
    xt = nc.dram_tensor("xt", [BLOC, H, S], F32R, kind="ExternalInput").ap()
    wa2t = nc.dram_tensor("wa2t", [H, H], F32R, kind="ExternalInput").ap()
    wa1t = nc.dram_tensor("wa1t", [H, H], F32R, kind="ExternalInput").ap()
    ht = nc.dram_tensor("ht", [H, BLOC], F32R, kind="ExternalInput").ap()
    # masked Ws^T layout: wstm[p, h*16+8] = Ws[h*128+p], else 0.  MM2 for
    # (h, b) uses the (128, 8) slice [h*16+8-b : h*16+16-b] whose only
    # nonzero column lands at position b -> scores write psum partition b.
    wstm = nc.dram_tensor("wstm", [128, 16 * HT], F32R, kind="ExternalInput").ap()
    ba = nc.dram_tensor("ba", [H, 1], F32, kind="ExternalInput").ap()
    ped = nc.dram_tensor("ped", [BLOC, S], F32, kind="ExternalInput").ap()
    msk = nc.dram_tensor("msk", [BLOC, S], U8, kind="ExternalInput").ap()
    outp = nc.dram_tensor("out", [BLOC, S], F32, kind="ExternalOutput").ap()

    with tile.TileContext(nc) as tc:
        with tc.tile_pool(name="wpool", bufs=1) as wpool, \
             tc.tile_pool(name="xpool", bufs=2) as xpool, \
             tc.tile_pool(name="epool", bufs=3) as epool, \
             tc.tile_pool(name="spool", bufs=1) as spool, \
             tc.tile_pool(name="eps", bufs=2, space="PSUM") as eps, \
             tc.tile_pool(name="sps", bufs=2, space="PSUM") as sps, \
             tc.tile_pool(name="cps", bufs=2, space="PSUM") as cps:

            # ---- resident weights / small tensors ----
            wa2_sb = []
            for k in range(KT):
                w = wpool.tile([128, H], F32R, tag=f"wa2_{k}")
                nc.sync.dma_start(w[:], wa2t[k * 128:(k + 1) * 128, :])
                wa2_sb.append(w)
            wstm_sb = wpool.tile([128, 16 * HT], F32R, tag="wstm")
            nc.sync.dma_start(wstm_sb[:], wstm)
            ba_sb = wpool.tile([128, HT], F32, tag="ba")
            nc.sync.dma_start(ba_sb[:], ba.rearrange("(k p) o -> p (k o)", p=128))
            ht_sb = []
            for k in range(KT):
                t = wpool.tile([128, BLOC], F32R, tag=f"ht_{k}")
                nc.sync.dma_start(t[:], ht[k * 128:(k + 1) * 128, :])
                ht_sb.append(t)
            wa1_sb = []
            for k in range(KT):
                w = wpool.tile([128, H], F32R, tag=f"wa1_{k}")
                nc.sync.dma_start(w[:], wa1t[k * 128:(k + 1) * 128, :])
                wa1_sb.append(w)

            # epilogue inputs
            ped_sb = spool.tile([BLOC, S], F32, tag="ped")
            nc.sync.dma_start(ped_sb[:], ped)
            msk_sb = spool.tile([BLOC, S], U8, tag="msk")
            nc.sync.dma_start(msk_sb[:], msk)
            negbig = spool.tile([BLOC, S], F32, tag="negbig")
            nc.vector.memset(negbig[:], -1e12)

            # ---- cT = Wa1 @ hidden^T + ba  -> (H, BLOC) as 8 tiles (128, BLOC)
            ct_sb = []
            for h in range(HT):
                cp = cps.tile([128, BLOC], F32, tag="cps")
                for k in range(KT):
                    nc.tensor.matmul(cp[:], wa1_sb[k][:, h * 128:(h + 1) * 128],
                                     ht_sb[k][:], start=(k == 0), stop=(k == KT - 1))
                ct = wpool.tile([128, BLOC], F32, tag=f"ct_{h}")
                nc.vector.tensor_scalar_add(ct[:], cp[:], ba_sb[:, h:h + 1])
                ct_sb.append(ct)

            # ---- main loop over local batches ----
            spsum = sps.tile([BLOC, S], F32, tag="sp")  # one bank, all scores
            deferred = None  # (h, e_tile, b)
            for b in range(BLOC):
                x_sb = []
                for k in range(KT):
                    x = xpool.tile([128, S], F32R, tag=f"xt_{k}")
                    nc.sync.dma_start(x[:], xt[b, k * 128:(k + 1) * 128, :])
                    x_sb.append(x)
                for h in range(HT):
                    ep = eps.tile([128, S], F32, tag="ep")
                    for k in range(KT):
                        nc.tensor.matmul(ep[:], wa2_sb[k][:, h * 128:(h + 1) * 128],
                                         x_sb[k][:], start=(k == 0),
                                         stop=(k == KT - 1))
                    e_sb = epool.tile([128, S], F32R, tag="e")
                    nc.scalar.activation(e_sb[:], ep[:], AF.Relu,
                                         bias=ct_sb[h][:, b:b + 1], scale=1.0)
                    # emit previous (b,h) score matmul now: keeps PE a full
                    # h-block ahead of the ACT dependency
                    if deferred is not None:
                        dh, de, db = deferred
                        nc.tensor.matmul(
                            spsum[:], wstm_sb[:, dh * 16 + 8 - db:dh * 16 + 16 - db],
                            de[:], start=(dh == 0 and db == 0),
                            stop=(dh == HT - 1 and db == BLOC - 1))
                    deferred = (h, e_sb, b)
            dh, de, db = deferred
            nc.tensor.matmul(spsum[:], wstm_sb[:, dh * 16 + 8 - db:dh * 16 + 16 - db],
                             de[:], start=(dh == 0 and db == 0),
                             stop=(dh == HT - 1 and db == BLOC - 1))

            # ---- epilogue: t = scores + pe/SCALE ; mask ; softmax(SCALE*t) ----
            t_sb = spool.tile([BLOC, S], F32, tag="t")
            nc.vector.tensor_tensor(out=t_sb[:], in0=spsum[:], in1=ped_sb[:],
                                    op=mybir.AluOpType.add)
            nc.vector.copy_predicated(t_sb[:], msk_sb[:], negbig[:])
            nmax = spool.tile([BLOC, 1], F32, tag="nmax")
            nc.vector.tensor_reduce(out=nmax[:], in_=t_sb[:],
                                    op=mybir.AluOpType.max,
                                    axis=mybir.AxisListType.X, negate=True)
            nmax_s = spool.tile([BLOC, 1], F32, tag="nmax_s")
            nc.vector.tensor_scalar_mul(nmax_s[:], nmax[:], SCALE)
            u_sb = spool.tile([BLOC, S], F32, tag="u")
            esum = spool.tile([BLOC, 1], F32, tag="esum")
            nc.scalar.activation(u_sb[:], t_sb[:], AF.Exp, bias=nmax_s[:],
                                 scale=SCALE, accum_out=esum[:])
            rcp = spool.tile([BLOC, 1], F32, tag="rcp")
            nc.vector.reciprocal(rcp[:], esum[:])
            o_sb = spool.tile([BLOC, S], F32, tag="o")
            nc.vector.tensor_scalar_mul(o_sb[:], u_sb[:], rcp[:])
            nc.sync.dma_start(outp, o_sb[:])

    nc.compile()
    return nc


def make_in_maps(hidden, encoder_outputs, pe, seq_mask, Wa, ba, Ws):
    """Host-side sharding + layout prep (transposes only, no math beyond pe/SCALE)."""
    hidden = np.asarray(hidden, dtype=np.float32)
    enc = np.asarray(encoder_outputs, dtype=np.float32)
    pe = np.asarray(pe, dtype=np.float32)
    seq_mask = np.asarray(seq_mask)
    Wa = np.asarray(Wa, dtype=np.float32)
    ba = np.asarray(ba, dtype=np.float32)
    Ws = np.asarray(Ws, dtype=np.float32)

    wa1t = np.ascontiguousarray(Wa[:, :H].T)          # (H, H) = (f, h)
    wa2t = np.ascontiguousarray(Wa[:, H:].T)          # (H, H)
    wstm = np.zeros((128, 16 * HT), dtype=np.float32)
    for h in range(HT):
        wstm[:, h * 16 + 8] = Ws[0, h * 128:(h + 1) * 128]
    ba_col = np.ascontiguousarray(ba.reshape(H, 1))
    ped_all = (pe / np.float32(SCALE)).astype(np.float32)
    msk_all = seq_mask.astype(np.uint8)

    in_maps = []
    for c in range(NCORES):
        bsl = slice(c * BLOC, (c + 1) * BLOC)
        xt = np.ascontiguousarray(enc[:, bsl, :].transpose(1, 2, 0))  # (BLOC,H,S)
        ht = np.ascontiguousarray(hidden[0, bsl, :].T)                # (H, BLOC)
        in_maps.append({
            "xt": xt, "wa2t": wa2t, "wa1t": wa1t, "ht": ht, "wstm": wstm,
            "ba": ba_col, "ped": np.ascontiguousarray(ped_all[bsl]),
            "msk": np.ascontiguousarray(msk_all[bsl]),
        })
    return in_maps


_NC_CACHE = None


def kernel(hidden, encoder_outputs, pe, seq_mask, Wa, ba, Ws):
    global _NC_CACHE
    if _NC_CACHE is None:
        _NC_CACHE = build_nc()
    nc = _NC_CACHE
    in_maps = make_in_maps(hidden, encoder_outputs, pe, seq_mask, Wa, ba, Ws)
    res = run_bass_kernel_spmd(nc, in_maps, list(range(NCORES)))
    attn = np.concatenate([res.results[c]["out"] for c in range(NCORES)], axis=0)
    return attn[:, None, :].astype(np.float32)
